# revision 3
# baseline (speedup 1.0000x reference)
"""Position-only MoE router kernel for Trainium2 (8 NeuronCores, SPMD).

Problem: x[8,2048,1024], tile_sigs[8,32], W[8,1024,1024], b[8,1024].
Routing idx[s] = argmax_t( pe[s] @ sign(tile_sigs[t]) ) depends only on the
position s, so it is computed on the host and baked into the schedule.

Strategy (expert-parallel, bf16):
  - Tokens from ALL batches are grouped by expert and spread over the 8
    cores so each core processes NT=17 tiles of 128 tokens split into NG=3
    fixed-size segments; each segment is single-expert, its weight is
    host-gathered per core.  One shared instruction stream; all per-core
    variation (which expert, which tokens) lives in the input data.
  - Everything on the wire is bf16: per core ~4.25MB tokens in, NG*2MB
    weights, 4.25MB out => ~14.5MB vs PE ~60us => PE-bound.
  - Bias is added on the host after the kernel (b[idx[s]] lookup), which
    removes the K=1 bias matmuls (512 PE cycles each) entirely.

Raw Bass (no Tile framework): explicit per-engine streams + semaphores.
  SP  : per-tile xt DMAs, per-tile y stores
  ACT : per-(segment,k-chunk) W loads (double-buffered slots)
  PE  : matmuls  out[tok,o] += xt[k,tok].T @ w[k,o]
  DVE : PSUM(f32) -> SBUF(bf16) output copies
"""

import math
import os
import sys

import numpy as np

for _p in ("/opt/trn_rl_repo", "/opt/trn_rl_repo/concourse"):
    if _p not in sys.path and os.path.isdir(_p):
        sys.path.append(_p)

B, S, D, T, P = 8, 2048, 1024, 8, 32
NCORES = 8
KC = D // 128  # 8 contraction chunks
WS = 2  # W double-buffer slots
PS = 3  # PSUM accumulator slots
OS = 3  # output staging slots

LAST_RESULTS = None  # BassKernelResults of the most recent run (for profiling)
_CACHE = {}


def _routing_idx(tile_sigs: np.ndarray) -> np.ndarray:
    pos = np.arange(S, dtype=np.float32)[:, None]
    div = np.exp(
        np.arange(0, P, 2, dtype=np.float32) * (-math.log(10000.0) / P)
    ).astype(np.float32)
    ang = pos * div
    pe = np.zeros((S, P), np.float32)
    pe[:, 0::2] = np.sin(ang)
    pe[:, 1::2] = np.cos(ang)
    scores = pe @ np.sign(tile_sigs).astype(np.float32).T
    return np.argmax(scores, axis=-1)


def _solve_assignment(counts, sizes):
    """Assign one expert to each of the 8*len(sizes) segments (8 cores with
    identical per-core segment sizes) so every expert e gets >= counts[e]
    tiles.  Returns {expert: [seg sizes]} or None."""
    caps = sorted([s for s in sizes for _ in range(NCORES)], reverse=True)
    slack = sum(caps) - int(sum(counts))
    if slack < 0:
        return None
    order = sorted(range(len(counts)), key=lambda e: -counts[e])
    best = None

    def rec(caps, ei, acc, slack_left):
        nonlocal best
        if best is not None:
            return
        if ei == len(order):
            if not caps:
                best = dict(acc)
            return
        e = order[ei]
        need = counts[e]
        if need == 0:
            rec(caps, ei + 1, acc, slack_left)
            return
        n = len(caps)

        def pick(i, chosen, ssum):
            if best is not None:
                return
            if ssum >= need:
                if ssum - need <= slack_left:
                    rem = list(caps)
                    for c in chosen:
                        rem.remove(c)
                    rec(
                        tuple(rem),
                        ei + 1,
                        acc + [(e, tuple(chosen))],
                        slack_left - (ssum - need),
                    )
                return
            if i == n or ssum + sum(caps[i:]) < need:
                return
            last = None
            for j in range(i, n):
                if caps[j] == last:
                    continue
                last = caps[j]
                pick(j + 1, chosen + [caps[j]], ssum + caps[j])

        pick(0, [], 0)

    rec(tuple(caps), 0, [], slack)
    return best


def _compositions(total, parts, lo=1):
    if parts == 1:
        if total >= lo:
            yield (total,)
        return
    for first in range(lo, total - (parts - 1) * lo + 1):
        for rest in _compositions(total - first, parts - 1, first):
            yield (first,) + rest


def _plan(idx: np.ndarray):
    """Build the global schedule.

    Returns (sizes, core_experts, core_tokens) where
      sizes        : per-core segment tile counts, descending program order
      core_experts : [NCORES][NG] expert id per segment
      core_tokens  : [NCORES] int32 [NT*128] global token ids (b*S + s)
    """
    counts = np.array(
        [int(np.ceil((idx == e).sum() * B / 128)) for e in range(T)]
    )
    total = int(counts.sum())
    assignment = None
    for nt in range(max(1, (total + NCORES - 1) // NCORES), total + 1):
        for ng in (2, 3, 4):
            for sizes in _compositions(nt, ng):
                assignment = _solve_assignment(counts, sizes)
                if assignment is not None:
                    break
            if assignment is not None:
                break
        if assignment is not None:
            break
    sizes = tuple(sorted(sizes, reverse=True))
    NG = len(sizes)

    # pack segments onto cores: expert -> multiset of segment sizes; each
    # core has one segment of each size in `sizes` (duplicates allowed).
    slots = {s: [] for s in set(sizes)}  # size -> [(core, seg_pos)]
    for c in range(NCORES):
        for g, s in enumerate(sizes):
            slots[s].append((c, g))
    core_experts = [[None] * NG for _ in range(NCORES)]
    for e, segs in sorted(assignment.items(), key=lambda kv: -counts[kv[0]]):
        for s in segs:
            c, g = slots[s].pop()
            core_experts[c][g] = e

    # token streams: expert token pool consumed across its segments in a
    # fixed global order; padding duplicates the last real token.
    pools = {}
    for e in range(T):
        pos_e = np.nonzero(idx == e)[0]
        if len(pos_e) == 0:
            pools[e] = np.zeros(0, dtype=np.int64)
            continue
        toks = (np.arange(B, dtype=np.int64)[:, None] * S + pos_e[None, :]).ravel()
        pools[e] = toks
    used = {e: 0 for e in range(T)}
    core_tokens = []
    for c in range(NCORES):
        parts = []
        for g, s in enumerate(sizes):
            e = core_experts[c][g]
            pool = pools[e]
            a = used[e]
            b_ = min(a + s * 128, len(pool))
            seg = pool[a:b_]
            used[e] = b_
            if len(seg) < s * 128:
                fill = pool[-1] if len(pool) else 0
                seg = np.concatenate(
                    [seg, np.full(s * 128 - len(seg), fill, dtype=np.int64)]
                )
            parts.append(seg)
        core_tokens.append(np.concatenate(parts))
    return sizes, core_experts, core_tokens


def _build_nc(NT: int, sizes: tuple):
    import concourse.bass as bass
    import concourse.mybir as mybir

    f32 = mybir.dt.float32
    bf16 = mybir.dt.bfloat16
    NG = len(sizes)
    # cumulative tile index at end of each segment, and tile -> segment map
    t_end = []
    acc = 0
    for s in sizes:
        acc += s
        t_end.append(acc)
    seg_of = []
    for g, s in enumerate(sizes):
        seg_of += [g] * s

    # xt arrives in staged chunks, one DMA + one semaphore each (a shared
    # counting semaphore across multiple in-flight DMAs is racy: the 16
    # engine-increments of independent DMAs interleave).
    xb = [0, 1, 3, min(9, NT), NT]
    xb = sorted(set(min(v, NT) for v in xb))
    x_chunks = list(zip(xb[:-1], xb[1:]))  # [(lo,hi)) tile ranges
    KH = KC // 2  # W segment 0 is split in two halves for early PE start

    nc = bass.Bass()
    # host layouts:
    #   xt [128, NT, KC, 128]   xt[p,t,k,m] = x_tok[t*128+m, k*128+p]
    #   wt [NG, 128, KC, 1024]  wt[g,p,k,o] = W[e_g][o, k*128+p]
    xt_d = nc.dram_tensor("xt", [128, NT, KC, 128], bf16, kind="ExternalInput")
    wt_d = nc.dram_tensor("wt", [NG, 128, KC, D], bf16, kind="ExternalInput")
    y_d = nc.dram_tensor("y", [NT * 128, D], bf16, kind="ExternalOutput")

    from contextlib import ExitStack

    with ExitStack() as ctx:
        xt_sb = ctx.enter_context(nc.sbuf_tensor([128, NT, KC, 128], bf16))
        w_sb = ctx.enter_context(nc.sbuf_tensor([128, WS, KC, D], bf16))
        out_sb = ctx.enter_context(nc.sbuf_tensor([128, OS, D], bf16))
        ps = ctx.enter_context(nc.psum_tensor([128, PS, D], f32))
        x_s = [
            ctx.enter_context(nc.semaphore(f"dma_x{i}"))
            for i in range(len(x_chunks))
        ]
        wha = ctx.enter_context(nc.semaphore("dma_wha"))
        whb = ctx.enter_context(nc.semaphore("dma_whb"))
        w_seg = [
            ctx.enter_context(nc.semaphore(f"dma_w{g}")) for g in range(1, NG)
        ]
        dma_y_s = [
            ctx.enter_context(nc.semaphore(f"dma_y{i}")) for i in range(OS)
        ]
        pe_t = ctx.enter_context(nc.semaphore("pe_t"))
        dve_c = ctx.enter_context(nc.semaphore("dve_c"))
        block = ctx.enter_context(nc.Block())

        y_count = [len(range(s, NT, OS)) for s in range(OS)]

        @block.sync
        def _(eng):
            for i, (lo, hi) in enumerate(x_chunks):
                if i == 1:
                    # keep startup HBM bandwidth for W segment 0
                    eng.wait_ge(whb, 16)
                eng.dma_start(
                    xt_sb[:, lo:hi, :, :], xt_d[:, lo:hi, :, :]
                ).then_inc(x_s[i], 16)
            for t in range(NT):
                eng.wait_ge(dve_c, t + 1)
                eng.dma_start(
                    y_d[t * 128 : (t + 1) * 128, :], out_sb[:, t % OS, :]
                ).then_inc(dma_y_s[t % OS], 16)
            for s in range(OS):
                eng.wait_ge(dma_y_s[s], 16 * y_count[s])

        @block.scalar
        def _(eng):
            eng.dma_start(w_sb[:, 0, 0:KH, :], wt_d[0, :, 0:KH, :]).then_inc(
                wha, 16
            )
            eng.dma_start(w_sb[:, 0, KH:KC, :], wt_d[0, :, KH:KC, :]).then_inc(
                whb, 16
            )
            for g in range(1, NG):
                if g >= WS:
                    eng.wait_ge(pe_t, t_end[g - WS])
                else:
                    # not needed for correctness: delays the prefetch so it
                    # doesn't steal startup bandwidth from W0/xt
                    eng.wait_ge(pe_t, min(5, t_end[0] - 1))
                eng.dma_start(w_sb[:, g % WS, :, :], wt_d[g]).then_inc(
                    w_seg[g - 1], 16
                )

        @block.tensor
        def _(eng):
            chunk_start = {lo: i for i, (lo, hi) in enumerate(x_chunks)}
            for t in range(NT):
                g = seg_of[t]
                slot = g % WS
                first_of_seg = t == 0 or seg_of[t - 1] != g
                if t in chunk_start:
                    eng.wait_ge(x_s[chunk_start[t]], 16)
                if t >= PS:
                    eng.wait_ge(dve_c, t - PS + 1)
                if first_of_seg and g > 0:
                    eng.wait_ge(w_seg[g - 1], 16)
                pslot = t % PS
                for k in range(KC):
                    if t == 0 and k == 0:
                        eng.wait_ge(wha, 16)
                    if t == 0 and k == KH:
                        eng.wait_ge(whb, 16)
                    lhsT = xt_sb[:, t, k, :]
                    for h in range(2):
                        mm = eng.matmul(
                            ps[:, pslot, h * 512 : (h + 1) * 512],
                            lhsT,
                            w_sb[:, slot, k, h * 512 : (h + 1) * 512],
                            start=(k == 0),
                            stop=(k == KC - 1),
                        )
                mm.then_inc(pe_t, 1)

        @block.vector
        def _(eng):
            for t in range(NT):
                eng.wait_ge(pe_t, t + 1)
                if t >= OS:
                    eng.wait_ge(dma_y_s[t % OS], 16 * ((t - OS) // OS + 1))
                eng.tensor_copy(out_sb[:, t % OS, :], ps[:, t % PS, :]).then_inc(
                    dve_c, 1
                )

    return nc


def kernel(x, tile_sigs, W, b):
    global LAST_RESULTS
    from concourse.bass_utils import run_bass_kernel_spmd
    from ml_dtypes import bfloat16

    x = np.asarray(x, dtype=np.float32)
    tile_sigs = np.asarray(tile_sigs, dtype=np.float32)
    W = np.asarray(W, dtype=np.float32)
    b = np.asarray(b, dtype=np.float32)

    idx = _routing_idx(tile_sigs)
    sizes, core_experts, core_tokens = _plan(idx)
    NT = sum(sizes)
    NG = len(sizes)

    key = (NT, sizes)
    if key in _CACHE:
        nc = _CACHE[key]
    else:
        nc = _build_nc(NT, sizes)
        _CACHE[key] = nc

    # host-side shard prep
    x_flat = x.reshape(B * S, D)
    wt_experts = {}
    for e in set(e for ce in core_experts for e in ce):
        # [128, KC, D]: wt[p,k,o] = W[e][o, k*128+p]
        wt_experts[e] = np.ascontiguousarray(
            W[e].T.reshape(KC, 128, D).transpose(1, 0, 2)
        ).astype(bfloat16)
    in_maps = []
    for c in range(NCORES):
        toks = core_tokens[c]
        xg = x_flat[toks]  # [NT*128, D] f32
        xt = np.ascontiguousarray(
            xg.reshape(NT, 128, KC, 128).transpose(3, 0, 2, 1)
        ).astype(bfloat16)
        wt = np.stack([wt_experts[e] for e in core_experts[c]])
        in_maps.append({"xt": xt, "wt": wt})

    core_ids = list(range(NCORES))
    res = run_bass_kernel_spmd(nc, in_maps, core_ids)
    LAST_RESULTS = res

    out_flat = np.empty((B * S, D), dtype=np.float32)
    for c in range(NCORES):
        yp = np.asarray(res.results[c]["y"]).astype(np.float32)
        out_flat[core_tokens[c]] = yp
    out = out_flat.reshape(B, S, D)
    out += b[idx][None, :, :]  # bias, host-side
    return out
